# revision 6
# baseline (speedup 1.0000x reference)
"""CompressiveDecoder forward on 8 TRN2 NeuronCores (Bass/Tile).

Sharding: core = 2*b + hf  (b = batch element 0..3, hf = query half 0..1).
Activations are feature-major xT [D, S] on-chip. Attention internals run in
bf16, FFN / output projections in float32r (tf32), residual stream in f32.
One pairwise bf16 AllGather of x per layer boundary. The Music-Transformer
relative-position skew is a strided DMA read from a DRAM staging buffer.
"""
import numpy as np
import ml_dtypes

N_LAYERS, B, S, D, H, DH = 4, 4, 512, 512, 8, 64
MEM, CMEM, RATIO, FFMUL, VOCAB, LLAT = 512, 128, 4, 4, 512, 256
KV = CMEM + MEM + S          # 1152
DF = D * FFMUL               # 2048
SH = S // 2                  # 256 local queries per core
NP = H // 2                  # 4 head pairs
PKV = KV + 256               # 1408 staged P width (rolled pos_emb)
STG_ROW = 1664               # staging row stride
STG_N = 128 * STG_ROW + 2048

_BUILT = None


def _tf32(x):
    b = np.ascontiguousarray(x, np.float32).view(np.uint32)
    b = (b + 0x1000) & 0xFFFFE000
    return np.ascontiguousarray(b.view(np.float32))


def _bf16(x):
    return np.ascontiguousarray(
        np.ascontiguousarray(x, np.float32).astype(ml_dtypes.bfloat16))


def _banks(L):
    return [(b, min(b + 512, L)) for b in range(0, L, 512)]


def _build():
    import contextlib
    import concourse.bacc as bacc
    import concourse.mybir as mybir
    import concourse.tile as tile

    F32 = mybir.dt.float32
    F32R = mybir.dt.float32r
    BF16 = mybir.dt.bfloat16
    AFT = mybir.ActivationFunctionType
    ALU = mybir.AluOpType

    nc = bacc.Bacc("TRN2", target_bir_lowering=False, debug=False,
                   num_devices=8)

    def din(name, shape, dt):
        return nc.dram_tensor(name, shape, dt, kind="ExternalInput")

    X0F = din("X0F", [D, S], BF16)
    X0L = din("X0L", [D, SH], F32)
    CMM = din("CMM", [N_LAYERS, D, CMEM + MEM], BF16)
    LAT = din("LAT", [D, LLAT], BF16)
    POSE = din("POSE", [NP, 128, PKV], BF16)
    WQ = din("WQ", [N_LAYERS, D, D], BF16)
    WK = din("WK", [N_LAYERS, D, D], BF16)
    WV = din("WV", [N_LAYERS, D, D], BF16)
    WO = din("WO", [N_LAYERS, D, D], F32R)
    SQw = din("SQw", [N_LAYERS, D, D], BF16)
    SKw = din("SKw", [N_LAYERS, D, D], BF16)
    SVw = din("SVw", [N_LAYERS, D, D], BF16)
    SOw = din("SOw", [N_LAYERS, D, D], F32R)
    CW = din("CW", [N_LAYERS, RATIO, D, D], BF16)
    CBv = din("CBv", [N_LAYERS, D], F32)
    W1 = din("W1", [N_LAYERS, D, DF], F32R)
    B1v = din("B1v", [N_LAYERS, DF], F32)
    W2 = din("W2", [N_LAYERS, DF, D], F32R)
    B2v = din("B2v", [N_LAYERS, D], F32)
    L1G = din("L1G", [N_LAYERS, D], F32)
    L1B = din("L1B", [N_LAYERS, D], F32)
    L2G = din("L2G", [N_LAYERS, D], F32)
    L2B = din("L2B", [N_LAYERS, D], F32)
    IDN = din("IDN", [128, 128], BF16)

    XOUT = nc.dram_tensor("XOUT", [D, SH], F32, kind="ExternalOutput")
    LOSS = nc.dram_tensor("LOSS", [128, 1], F32, kind="ExternalOutput")

    with tile.TileContext(nc) as tc:
        ctx = contextlib.ExitStack()
        sb = ctx.enter_context(tc.tile_pool(name="sb", bufs=2))
        sbc = ctx.enter_context(tc.tile_pool(name="sbc", bufs=1))
        sbw1 = ctx.enter_context(tc.tile_pool(name="sbw1", bufs=2))
        psu = ctx.enter_context(tc.tile_pool(name="psu", bufs=6, space="PSUM"))
        pst = ctx.enter_context(tc.tile_pool(name="pst", bufs=2, space="PSUM"))
        dram = ctx.enter_context(tc.tile_pool(name="dram", bufs=1,
                                              space="DRAM"))

        def PS():
            return psu.tile([128, 512], F32, tag="u", name="ups")

        # ---- constants ----
        idn = sbc.tile([128, 128], BF16, tag="idn")
        nc.sync.dma_start(out=idn[:], in_=IDN[:])
        ones = sbc.tile([128, 1], BF16, tag="ones")
        nc.any.memset(ones[:], 1.0)
        eps = sbc.tile([1, 1], F32, tag="eps")
        nc.any.memset(eps[:], 1e-5)
        loss_acc = sbc.tile([128, 1], F32, tag="loss_acc")
        nc.any.memset(loss_acc[:], 0.0)
        pose_t = []
        for p in range(NP):
            t = sbc.tile([128, PKV], BF16, tag=f"pose{p}")
            nc.sync.dma_start(out=t[:], in_=POSE[p])
            pose_t.append(t)

        zt = sb.tile([128, 512], BF16, tag="zt", bufs=1)
        nc.any.memset(zt[:], 0.0)
        stages = []
        for i in range(4):
            st = dram.tile([STG_N], BF16, tag=f"stage{i}")
            v = st[:][0:128 * STG_ROW].rearrange("(p x) -> p x", x=STG_ROW)
            for z0 in range(0, STG_ROW, 512):
                z1 = min(z0 + 512, STG_ROW)
                nc.sync.dma_start(out=v[:, z0:z1], in_=zt[:, 0:z1 - z0])
            stages.append(st)

        ag_in, ag_out = [], []
        for l in range(N_LAYERS - 1):
            ag_in.append(dram.tile([D * SH], BF16, tag=f"agi{l}", name=f"agi{l}"))
            ag_out.append(dram.tile([2 * D * SH], BF16, tag=f"ago{l}", name=f"ago{l}"))

        # ---- initial activations ----
        xf, xl = [], []
        for c in range(4):
            t = sb.tile([128, S], BF16, tag=f"xf{c}", bufs=1)
            nc.sync.dma_start(out=t[:], in_=X0F[c * 128:(c + 1) * 128, :])
            xf.append(t)
            t2 = sb.tile([128, SH], F32, tag=f"xl{c}")
            nc.sync.dma_start(out=t2[:], in_=X0L[c * 128:(c + 1) * 128, :])
            xl.append(t2)

        def load_w(W, l, tag, dt):
            ts = []
            for c in range(4):
                t = sbc.tile([128, D], dt, tag=f"{tag}{c}")
                nc.sync.dma_start(out=t[:], in_=W[l, c * 128:(c + 1) * 128, :])
                ts.append(t)
            return ts

        def col_vec(W, l, c, tag, n=128):
            t = sb.tile([n, 1], F32, tag=tag)
            nc.sync.dma_start(out=t[:],
                              in_=W[l, c * n:(c + 1) * n].unsqueeze(1))
            return t

        def proj_pairs(w_tiles, rhs_tiles, ncols, tag, rhs_cols=None):
            outs = []
            for p in range(NP):
                ps = PS()
                for c in range(4):
                    rhs = (rhs_tiles[c][:] if rhs_cols is None
                           else rhs_tiles[c][:, rhs_cols])
                    nc.tensor.matmul(ps[:, 0:ncols],
                                     w_tiles[c][:, p * 128:(p + 1) * 128],
                                     rhs, start=(c == 0), stop=(c == 3))
                o = sb.tile([128, ncols], BF16, tag=f"po_{tag}{p}", bufs=1)
                nc.vector.tensor_copy(o[:], ps[:, 0:ncols])
                outs.append(o)
            return outs

        def kt_wide(w_tiles, srcs, tag):
            """k^T pair tiles [128, 1152] bf16; srcs: (tiles, col0, col1, kv0)."""
            outs = []
            for p in range(NP):
                o = sb.tile([128, KV], BF16, tag=f"kt_{tag}{p}", bufs=1)
                for b0, b1 in _banks(KV):
                    ps = PS()
                    segs = []
                    for (tl, c0, c1, kv0) in srcs:
                        lo = max(kv0, b0)
                        hi = min(kv0 + (c1 - c0), b1)
                        if lo < hi:
                            segs.append((tl, c0 + lo - kv0, c0 + hi - kv0,
                                         lo - b0))
                    for c in range(4):
                        for (tl, s0, s1, oo) in segs:
                            nc.tensor.matmul(
                                ps[:, oo:oo + s1 - s0],
                                w_tiles[c][:, p * 128:(p + 1) * 128],
                                tl[c][:, s0:s1],
                                start=(c == 0), stop=(c == 3))
                    nc.vector.tensor_copy(o[:, b0:b1], ps[:, 0:b1 - b0])
                outs.append(o)
            return outs

        def v_kvmajor(w_tiles, srcs, nkv, tag):
            outs = [[] for _ in range(NP)]
            for ch in range(nkv):
                kv0 = ch * 128
                for (tl, c0, c1, base) in srcs:
                    if base <= kv0 < base + (c1 - c0):
                        src_t, src_c = tl, c0 + kv0 - base
                        break
                for p in range(NP):
                    ps = PS()
                    for c in range(4):
                        nc.tensor.matmul(
                            ps[:, 0:128],
                            src_t[c][:, src_c:src_c + 128],
                            w_tiles[c][:, p * 128:(p + 1) * 128],
                            start=(c == 0), stop=(c == 3))
                    o = sb.tile([128, 128], BF16, tag=f"v_{tag}{p}_{ch}", bufs=1)
                    nc.vector.tensor_copy(o[:], ps[:, 0:128])
                    outs[p].append(o)
            return outs

        def pos_tiles(h, sc, qt_pair, r0):
            """Stage P = q_h . E''_h and skew-read shifted bank tiles."""
            st = stages[(h * 2 + sc) % 4]
            wview = st[:][0:128 * STG_ROW].rearrange("(p x) -> p x",
                                                     x=STG_ROW)
            for (b0, b1) in _banks(PKV):
                ps = PS()
                nc.tensor.matmul(ps[:, 0:b1 - b0],
                                 qt_pair[r0:r0 + 64, sc * 128:(sc + 1) * 128],
                                 pose_t[h // 2][r0:r0 + 64, b0:b1],
                                 start=True, stop=True)
                pbf = sb.tile([128, 512], BF16, tag="pbf")
                nc.scalar.activation(pbf[:, 0:b1 - b0], ps[:, 0:b1 - b0],
                                     AFT.Copy)
                nc.sync.dma_start(out=wview[:, b0:b1],
                                  in_=pbf[:, 0:b1 - b0])
            off = 511 - sc * 128
            rview = st[:][off:off + 128 * (STG_ROW - 1)].rearrange(
                "(p x) -> p x", x=STG_ROW - 1)
            outs = []
            for i, (b0, b1) in enumerate(_banks(KV)):
                pt = sb.tile([128, 512], BF16, tag=f"post{i}", bufs=1)
                nc.sync.dma_start(out=pt[:, 0:b1 - b0], in_=rview[:, b0:b1])
                outs.append(pt)
            return outs

        def attention(qt, kt, vt, L, out_dt, tag, with_pos=False):
            nkv = (L + 127) // 128
            bks = _banks(L)
            av_out = []
            for p in range(NP):
                av_ps = PS()
                for half in range(2):
                    h = 2 * p + half
                    r0 = half * 64
                    attnT = [sb.tile([128, SH], BF16, tag=f"aT_{tag}{ch}",
                                     name=f"aT{ch}", bufs=1)
                             for ch in range(nkv)]
                    for sc in range(2):
                        probs, dens = [], []
                        pos_ts = (pos_tiles(h, sc, qt[p], r0)
                                  if with_pos else None)
                        for bi, (b0, b1) in enumerate(bks):
                            w = b1 - b0
                            sc_ps = PS()
                            nc.tensor.matmul(
                                sc_ps[:, 0:w],
                                qt[p][r0:r0 + 64, sc * 128:(sc + 1) * 128],
                                kt[p][r0:r0 + 64, b0:b1],
                                start=True, stop=True)
                            if with_pos:
                                nc.vector.tensor_add(sc_ps[:, 0:w],
                                                     sc_ps[:, 0:w],
                                                     pos_ts[bi][:, 0:w])
                            pr = sb.tile([128, 512], BF16, tag=f"pr{bi}", bufs=1)
                            dn = sb.tile([128, 1], F32, tag=f"dn{bi}")
                            nc.scalar.activation(pr[:, 0:w], sc_ps[:, 0:w],
                                                 AFT.Exp, accum_out=dn[:])
                            probs.append(pr)
                            dens.append(dn)
                        den = dens[0]
                        for dn in dens[1:]:
                            nc.vector.tensor_add(den[:], den[:], dn[:])
                        rec = sb.tile([128, 1], F32, tag="rec")
                        nc.vector.reciprocal(rec[:], den[:])
                        for bi, (b0, b1) in enumerate(bks):
                            w = b1 - b0
                            pn = sb.tile([128, 512], BF16, tag=f"pn{bi}")
                            nc.scalar.activation(pn[:, 0:w], probs[bi][:, 0:w],
                                                 AFT.Copy, scale=rec[:])
                            for ch in range(b0 // 128, (b1 + 127) // 128):
                                c0 = ch * 128 - b0
                                cw = min(128, w - c0)
                                tp = pst.tile([128, 128], BF16, tag="tp", name="tp")
                                nc.tensor.transpose(tp[0:cw, :],
                                                    pn[:, c0:c0 + cw], idn[:])
                                nc.vector.tensor_copy(
                                    attnT[ch][0:cw, sc * 128:(sc + 1) * 128],
                                    tp[0:cw, :])
                    for ch in range(nkv):
                        cw = min(128, L - ch * 128)
                        nc.tensor.matmul(
                            av_ps[r0:r0 + 64, 0:SH],
                            vt[p][ch][0:cw, r0:r0 + 64],
                            attnT[ch][0:cw, :],
                            start=(ch == 0), stop=(ch == nkv - 1),
                            tile_position=(0, r0))
                o = sb.tile([128, SH], out_dt, tag=f"ao_{tag}{p}", bufs=1)
                nc.vector.tensor_copy(o[:], av_ps[:, 0:SH])
                av_out.append(o)
            return av_out

        def layernorm(pre, g_ts, b_ts, out_dt, tag):
            sxp, sqp = PS(), PS()
            for c in range(4):
                bft = sb.tile([128, SH], BF16, tag="lnb", bufs=1)
                nc.scalar.activation(bft[:], pre[c][:], AFT.Copy)
                sqt = sb.tile([128, SH], BF16, tag="lnq", bufs=1)
                nc.scalar.activation(sqt[:], pre[c][:], AFT.Square)
                nc.tensor.matmul(sxp[0:1, 0:SH], ones[:], bft[:],
                                 start=(c == 0), stop=(c == 3))
                nc.tensor.matmul(sqp[0:1, 0:SH], ones[:], sqt[:],
                                 start=(c == 0), stop=(c == 3))
            mean = sb.tile([1, SH], F32, tag="mn")
            nc.vector.tensor_scalar_mul(mean[:], sxp[0:1, 0:SH], 1.0 / D)
            ex2 = sb.tile([1, SH], F32, tag="e2")
            nc.vector.tensor_scalar_mul(ex2[:], sqp[0:1, 0:SH], 1.0 / D)
            m2 = sb.tile([1, SH], F32, tag="m2")
            nc.vector.tensor_mul(m2[:], mean[:], mean[:])
            var = sb.tile([1, SH], F32, tag="vr")
            nc.vector.tensor_sub(var[:], ex2[:], m2[:])
            std = sb.tile([1, SH], F32, tag="sd")
            nc.scalar.activation(std[:], var[:], AFT.Sqrt, bias=eps[:])
            rstd = sb.tile([1, SH], F32, tag="rs")
            nc.vector.reciprocal(rstd[:], std[:])
            mean_b = sb.tile([128, SH], F32, tag="mb", bufs=1)
            nc.gpsimd.partition_broadcast(mean_b[:], mean[:])
            rstd_b = sb.tile([128, SH], F32, tag="rb", bufs=1)
            nc.gpsimd.partition_broadcast(rstd_b[:], rstd[:])
            outs = []
            for c in range(4):
                t1 = sb.tile([128, SH], F32, tag="t1", bufs=1)
                nc.vector.tensor_sub(t1[:], pre[c][:], mean_b[:])
                t2 = sb.tile([128, SH], F32, tag="t2", bufs=1)
                nc.vector.tensor_mul(t2[:], t1[:], rstd_b[:])
                o = sb.tile([128, SH], out_dt, tag=f"lno_{tag}{c}", bufs=1)
                nc.vector.tensor_scalar(o[:], t2[:], g_ts[c][:], b_ts[c][:],
                                        ALU.mult, ALU.add)
                outs.append(o)
            return outs

        # ================= layers =================
        for l in range(N_LAYERS):
            wq = load_w(WQ, l, "wq", BF16)
            wk = load_w(WK, l, "wk", BF16)
            wv = load_w(WV, l, "wv", BF16)
            wo = load_w(WO, l, "wo", F32R)
            cmm = []
            for c in range(4):
                t = sbc.tile([128, CMEM + MEM], BF16, tag=f"cmm{c}")
                nc.sync.dma_start(out=t[:],
                                  in_=CMM[l, c * 128:(c + 1) * 128, :])
                cmm.append(t)
            kv_srcs = [(cmm, 0, 640, 0), (xf, 0, 512, 640)]
            mem_srcs = [(cmm, 128, 640, 0)]

            # ---- self attention ----
            xlb = []
            for c in range(4):
                t = sb.tile([128, SH], BF16, tag=f"xlb{c}", bufs=1)
                nc.scalar.activation(t[:], xl[c][:], AFT.Copy)
                xlb.append(t)
            qt = proj_pairs(wq, xlb, SH, "q")
            kt = kt_wide(wk, kv_srcs, "k")
            vt = v_kvmajor(wv, kv_srcs, 9, "v")
            att = attention(qt, kt, vt, KV, F32R, "sf", with_pos=True)

            a_pre = []
            for c in range(4):
                ps = PS()
                for k in range(4):
                    nc.tensor.matmul(ps[:, 0:SH],
                                     wo[k][:, c * 128:(c + 1) * 128],
                                     att[k][:], start=(k == 0), stop=(k == 3))
                t = sb.tile([128, SH], F32, tag=f"apre{c}", bufs=1)
                nc.vector.tensor_add(t[:], ps[:, 0:SH], xl[c][:])
                a_pre.append(t)
            g1 = [col_vec(L1G, l, c, f"l1g{c}") for c in range(4)]
            bb1 = [col_vec(L1B, l, c, f"l1b{c}") for c in range(4)]
            aT = layernorm(a_pre, g1, bb1, F32, "ln1")
            aTb = []
            for c in range(4):
                t = sb.tile([128, SH], BF16, tag=f"aTb{c}", bufs=1)
                nc.scalar.activation(t[:], aT[c][:], AFT.Copy)
                aTb.append(t)

            # ---- conv compress ----
            mem_lr = [cmm[c][:, 128:640].rearrange("p (a r) -> p a r", r=4)
                      for c in range(4)]
            cv_ps = [PS() for _ in range(4)]
            for r in range(RATIO):
                cwt = []
                for c in range(4):
                    t = sb.tile([128, D], BF16, tag=f"cw{c}")
                    nc.sync.dma_start(
                        out=t[:], in_=CW[l, r, c * 128:(c + 1) * 128, :])
                    cwt.append(t)
                for oc in range(4):
                    for c in range(4):
                        nc.tensor.matmul(
                            cv_ps[oc][:, 0:CMEM],
                            cwt[c][:, oc * 128:(oc + 1) * 128],
                            mem_lr[c][:, :, r],
                            start=(r == 0 and c == 0),
                            stop=(r == 3 and c == 3))
            ncm = []
            for oc in range(4):
                cb = col_vec(CBv, l, oc, f"cb{oc}")
                t = sb.tile([128, CMEM], BF16, tag=f"ncm{oc}")
                nc.vector.tensor_scalar_add(t[:], cv_ps[oc][:, 0:CMEM], cb[:])
                ncm.append(t)

            # ---- reconstruction attention loss ----
            qr = proj_pairs(wq, aTb, SH, "qr")
            kt_o = proj_pairs(wk, cmm, MEM, "ko", rhs_cols=slice(128, 640))
            vt_o = v_kvmajor(wv, mem_srcs, 4, "vo")
            tgt = attention(qr, kt_o, vt_o, MEM, F32, "ro")
            kt_n = proj_pairs(wk, ncm, CMEM, "kn")
            vt_n = v_kvmajor(wv, [(ncm, 0, CMEM, 0)], 1, "vn")
            rcn = attention(qr, kt_n, vt_n, CMEM, F32, "rn")
            for p in range(NP):
                df = sb.tile([128, SH], F32, tag="ldf")
                nc.vector.tensor_sub(df[:], rcn[p][:], tgt[p][:])
                trash = sb.tile([128, SH], F32, tag="ltr")
                lacc = sb.tile([128, 1], F32, tag="lac")
                nc.scalar.activation(trash[:], df[:], AFT.Square,
                                     accum_out=lacc[:])
                nc.vector.tensor_add(loss_acc[:], loss_acc[:], lacc[:])

            # ---- cross attention ----
            sq = load_w(SQw, l, "wq", BF16)
            sk = load_w(SKw, l, "wk", BF16)
            sv = load_w(SVw, l, "wv", BF16)
            so = load_w(SOw, l, "wo", F32R)
            lat = []
            for c in range(4):
                t = sbc.tile([128, LLAT], BF16, tag=f"lat{c}")
                nc.sync.dma_start(out=t[:], in_=LAT[c * 128:(c + 1) * 128, :])
                lat.append(t)
            qc = proj_pairs(sq, aTb, SH, "qc")
            kc = proj_pairs(sk, lat, LLAT, "kc")
            vc = v_kvmajor(sv, [(lat, 0, LLAT, 0)], 2, "vc")
            attc = attention(qc, kc, vc, LLAT, F32R, "cr")
            x_cross = []
            for c in range(4):
                ps = PS()
                for k in range(4):
                    nc.tensor.matmul(ps[:, 0:SH],
                                     so[k][:, c * 128:(c + 1) * 128],
                                     attc[k][:], start=(k == 0), stop=(k == 3))
                t = sb.tile([128, SH], F32, tag=f"xc{c}", bufs=1)
                nc.vector.tensor_copy(t[:], ps[:, 0:SH])
                x_cross.append(t)

            # ---- FFN ----
            g2 = [col_vec(L2G, l, c, f"l2g{c}") for c in range(4)]
            bb2 = [col_vec(L2B, l, c, f"l2b{c}") for c in range(4)]
            yT = layernorm(x_cross, g2, bb2, F32R, "ln2")
            w2ps = [PS() for _ in range(4)]
            for blk in range(4):
                w1t = []
                for c in range(4):
                    t = sbw1.tile([128, 512], F32R, tag=f"w1_{c}", bufs=1)
                    nc.sync.dma_start(
                        out=t[:],
                        in_=W1[l, c * 128:(c + 1) * 128,
                               blk * 512:(blk + 1) * 512])
                    w1t.append(t)
                for sub in range(4):
                    dfc = blk * 4 + sub
                    hps = PS()
                    for c in range(4):
                        nc.tensor.matmul(
                            hps[:, 0:SH],
                            w1t[c][:, sub * 128:(sub + 1) * 128],
                            yT[c][:], start=(c == 0), stop=(c == 3))
                    b1c = col_vec(B1v, l, dfc, "b1c")
                    gt = sb.tile([128, SH], F32R, tag="gt")
                    nc.scalar.activation(gt[:], hps[:, 0:SH], AFT.Gelu,
                                         bias=b1c[:])
                    w2t = sbw1.tile([128, D], F32R, tag="w2t")
                    nc.sync.dma_start(
                        out=w2t[:], in_=W2[l, dfc * 128:(dfc + 1) * 128, :])
                    for c in range(4):
                        nc.tensor.matmul(
                            w2ps[c][:, 0:SH],
                            w2t[:, c * 128:(c + 1) * 128], gt[:],
                            start=(dfc == 0), stop=(dfc == 15))
            x_new = []
            for c in range(4):
                b2c = col_vec(B2v, l, c, f"b2c{c}")
                t0 = sb.tile([128, SH], F32, tag="xn0", bufs=1)
                nc.vector.tensor_add(t0[:], w2ps[c][:, 0:SH], x_cross[c][:])
                t = sb.tile([128, SH], F32, tag=f"xl{c}")
                nc.vector.tensor_scalar_add(t[:], t0[:], b2c[:])
                x_new.append(t)
            xl = x_new

            # ---- AllGather x for next layer ----
            if l < N_LAYERS - 1:
                agi_v = ag_in[l][:].rearrange("(p x) -> p x", x=SH)
                for c in range(4):
                    t = sb.tile([128, SH], BF16, tag=f"xnb{c}")
                    nc.scalar.activation(t[:], x_new[c][:], AFT.Copy)
                    nc.sync.dma_start(out=agi_v[c * 128:(c + 1) * 128, :],
                                      in_=t[:])
                nc.gpsimd.collective_compute(
                    "AllGather", ALU.bypass,
                    replica_groups=[[0, 1], [2, 3], [4, 5], [6, 7]],
                    ins=[ag_in[l].opt()],
                    outs=[ag_out[l].opt()],
                )
                ago_v = ag_out[l][:].rearrange("(p x) -> p x", x=SH)
                xf = []
                for c in range(4):
                    t = sb.tile([128, S], BF16, tag=f"xf{c}", bufs=1)
                    nc.sync.dma_start(out=t[:, 0:SH],
                                      in_=ago_v[c * 128:(c + 1) * 128, :])
                    nc.sync.dma_start(
                        out=t[:, SH:S],
                        in_=ago_v[512 + c * 128:512 + (c + 1) * 128, :])
                    xf.append(t)

        for c in range(4):
            nc.sync.dma_start(out=XOUT[c * 128:(c + 1) * 128, :],
                              in_=xl[c][:])
        nc.sync.dma_start(out=LOSS[:], in_=loss_acc[:])
        ctx.close()

    nc.compile()
    return nc


def _prep(inputs):
    trg = np.asarray(inputs["trg"])
    latent = np.asarray(inputs["latent"], np.float32)
    mems = np.asarray(inputs["mems"], np.float32)
    cmems = np.asarray(inputs["cmems"], np.float32)
    pos_emb = np.asarray(inputs["pos_emb"], np.float32)
    embed = np.asarray(inputs["embed"], np.float32)
    W_self = np.asarray(inputs["W_self"], np.float32)
    W_src = np.asarray(inputs["W_src"], np.float32)
    conv_w = np.asarray(inputs["conv_w"], np.float32)

    assert np.asarray(inputs["trg_mask"]).all(), "masks must be all-True"
    assert np.asarray(inputs["src_mask"]).all(), "masks must be all-True"

    x0 = embed[trg]  # [B, S, D]
    shared = {
        "WQ": _bf16(W_self[:, 0] * 0.125),
        "WK": _bf16(W_self[:, 1]),
        "WV": _bf16(W_self[:, 2]),
        "WO": _tf32(W_self[:, 3]),
        "SQw": _bf16(W_src[:, 0] * 0.125),
        "SKw": _bf16(W_src[:, 1]),
        "SVw": _bf16(W_src[:, 2]),
        "SOw": _tf32(W_src[:, 3]),
        "CW": _bf16(conv_w.transpose(0, 3, 2, 1)),
        "CBv": np.ascontiguousarray(inputs["conv_b"], dtype=np.float32),
        "W1": _tf32(np.asarray(inputs["w1"])),
        "B1v": np.ascontiguousarray(inputs["b1"], dtype=np.float32),
        "W2": _tf32(np.asarray(inputs["w2"])),
        "B2v": np.ascontiguousarray(inputs["b2"], dtype=np.float32),
        "L1G": np.ascontiguousarray(inputs["ln1_g"], dtype=np.float32),
        "L1B": np.ascontiguousarray(inputs["ln1_b"], dtype=np.float32),
        "L2G": np.ascontiguousarray(inputs["ln2_g"], dtype=np.float32),
        "L2B": np.ascontiguousarray(inputs["ln2_b"], dtype=np.float32),
        "IDN": _bf16(np.eye(128, dtype=np.float32)),
    }
    pe_all = pos_emb.transpose(0, 2, 1) * 64.0   # [H, 64, KV]
    in_maps = []
    for core in range(8):
        b, hf = core // 2, core % 2
        # E'' = [zeros(hf*256) | E] padded to PKV cols (fixed-offset skew read)
        pe = np.zeros((H, DH, PKV), np.float32)
        pe[:, :, hf * 256:hf * 256 + KV] = pe_all
        pose = np.zeros((NP, 128, PKV), np.float32)
        for p in range(NP):
            pose[p, 0:64] = pe[2 * p]
            pose[p, 64:128] = pe[2 * p + 1]
        cmm = np.concatenate([cmems[:, b], mems[:, b]], axis=1)  # [L,640,D]
        m = {
            "X0F": _bf16(x0[b].T),
            "X0L": np.ascontiguousarray(x0[b].T[:, hf * SH:(hf + 1) * SH],
                                        dtype=np.float32),
            "CMM": _bf16(cmm.transpose(0, 2, 1)),
            "LAT": _bf16(latent[b].T),
            "POSE": _bf16(pose),
            **shared,
        }
        in_maps.append(m)
    return in_maps


def kernel(**inputs):
    global _BUILT
    from concourse.bass_utils import run_bass_kernel_spmd
    if _BUILT is None:
        _BUILT = _build()
    in_maps = _prep(inputs)
    res = run_bass_kernel_spmd(_BUILT, in_maps, list(range(8)))
    x = np.zeros((B, S, D), np.float32)
    loss = 0.0
    for core in range(8):
        b, hf = core // 2, core % 2
        r = res.results[core]
        x[b, hf * SH:(hf + 1) * SH, :] = r["XOUT"].T
        loss += float(r["LOSS"].sum())
    loss = np.float32(loss / (B * H * S * DH) / N_LAYERS)
    return x, loss


if __name__ == "__main__":
    d = np.load("/tmp/refin.npz")
    x, loss = kernel(**{k: d[k] for k in d.files})
    ref = np.load("/tmp/ref.npz")
    print("x relerr:", float(np.abs(x - ref["x"]).max() / np.abs(ref["x"]).max()))
    print("loss:", loss, "ref:", float(ref["loss"]), "relerr:",
          float(abs(loss - ref["loss"]) / abs(ref["loss"])))
